# revision 6
# baseline (speedup 1.0000x reference)
"""3x3 median filter (reflect padding) on Trainium2, 8-core data parallel.

Layout (per core, 4 images):
  partition p = ws*32 + hs*4 + b
    ws in 0..3  : vertical strip of 56 output columns
    hs in 0..7  : horizontal slice of 28 output rows
    b  in 0..3  : image index within the core's batch shard
  Each partition holds a "slab" of (CH+2) rows x (56+2) px x 3 ch = f32
  in the free dimension, so both vertical (stride F) and horizontal
  (stride C) neighbor access are free-dim offsets.

Median of 9 = med3( max3(col_lows), med3(col_meds), min3(col_highs) )
where each vertical column triple is sorted once and shared by the three
horizontally adjacent windows.
"""

import sys

if "/opt/trn_rl_repo" not in sys.path:
    sys.path.insert(0, "/opt/trn_rl_repo")

import numpy as np

import concourse.bass as bass  # noqa: F401
import concourse.tile as tile
from concourse import bacc, mybir
from concourse.ap import AP
from concourse.bass_utils import run_bass_kernel_spmd

F32 = mybir.dt.float32
MIN = mybir.AluOpType.min
MAX = mybir.AluOpType.max

B, H, W, C = 32, 224, 224, 3
NCORES = 8
BPC = B // NCORES  # 4 images per core
NW, WS = 4, 56     # strips per image row, strip width (output px)
NH, HS = 8, 28     # h-slices, rows per slice
CH = 14            # chunk rows (per compute iteration)
NCHUNK = HS // CH  # 2
SR = CH + 2        # slab rows incl. vertical halo
F = (WS + 2) * C   # 174 floats per slab row incl. horizontal halo
WC = W * C         # 672
IMG = H * WC       # elements per image
SC = WS * C        # 168 output floats per strip row

_CACHE = {}


def _src_cols(ws):
    """(src_float_offset, width, dst_float_offset) for strip ws."""
    if ws == 0:
        return 0, F - 3, 3
    if ws == NW - 1:
        return ws * SC - 3, F - 3, 0
    return ws * SC - 3, F, 0


def _build_kernel(tc, y, x):
    nc = tc.nc

    with tc.tile_pool(name="sb", bufs=1) as sb:
        for chunk in range(NCHUNK):
            S = sb.tile([128, SR, F], F32, tag="s", bufs=2, name=f"S{chunk}")

            # ---- loads (per (ws,hs): contiguous 4-partition block) -----
            for ws in range(NW):
                so, fw, do = _src_cols(ws)
                for hs in range(NH):
                    p0 = ws * 32 + hs * 4
                    r0 = hs * HS + chunk * CH - 1
                    if r0 < 0:
                        # rows 0..SR-2 into slab rows 1..SR-1, reflect row 1
                        esrc = AP(x.tensor, so,
                                  [[IMG, BPC], [WC, SR - 1], [1, fw]])
                        nc.sync.dma_start(S[p0:p0 + 4, 1:SR, do:do + fw], esrc)
                        rsrc = AP(x.tensor, WC + so, [[IMG, BPC], [1, fw]])
                        nc.sync.dma_start(S[p0:p0 + 4, 0:1, do:do + fw], rsrc)
                    elif r0 + SR > H:
                        esrc = AP(x.tensor, r0 * WC + so,
                                  [[IMG, BPC], [WC, SR - 1], [1, fw]])
                        nc.sync.dma_start(S[p0:p0 + 4, 0:SR - 1, do:do + fw], esrc)
                        rsrc = AP(x.tensor, (H - 2) * WC + so,
                                  [[IMG, BPC], [1, fw]])
                        nc.sync.dma_start(
                            S[p0:p0 + 4, SR - 1:SR, do:do + fw], rsrc)
                    else:
                        src = AP(x.tensor, r0 * WC + so,
                                 [[IMG, BPC], [WC, SR], [1, fw]])
                        nc.sync.dma_start(S[p0:p0 + 4, :, do:do + fw], src)

            # horizontal reflect: col -1 -> col 1 ; col W -> col W-2
            nc.vector.tensor_copy(S[0:32, :, 0:3], S[0:32, :, 6:9])
            nc.vector.tensor_copy(S[96:128, :, F - 3:F], S[96:128, :, F - 9:F - 6])

            # ---- stage 1: vertical column sort -------------------------
            P = sb.tile([128, SR - 1, F], F32, tag="p", name=f"P{chunk}")
            Q = sb.tile([128, SR - 1, F], F32, tag="q", name=f"Q{chunk}")
            nc.vector.tensor_tensor(P[:], S[:, 0:SR - 1, :], S[:, 1:SR, :], MIN)
            nc.vector.tensor_tensor(Q[:], S[:, 0:SR - 1, :], S[:, 1:SR, :], MAX)

            LO = sb.tile([128, CH, F], F32, tag="lo", name=f"LO{chunk}")
            T = sb.tile([128, CH, F], F32, tag="t", name=f"T{chunk}")
            HI = sb.tile([128, CH, F], F32, tag="hi", name=f"HI{chunk}")
            nc.vector.tensor_tensor(LO[:], P[:, 0:CH, :], S[:, 2:SR, :], MIN)
            nc.vector.tensor_tensor(T[:], Q[:, 0:CH, :], S[:, 2:SR, :], MIN)
            # MED (in T): max(P, min(Q, S+2))
            nc.vector.tensor_tensor(T[:], P[:, 0:CH, :], T[:], MAX)
            nc.vector.tensor_tensor(HI[:], Q[:, 0:CH, :], S[:, 2:SR, :], MAX)

            # ---- stage 2: horizontal merge -----------------------------
            U = sb.tile([128, CH, F - 3], F32, tag="uv", name=f"U{chunk}")
            A = sb.tile([128, CH, SC], F32, tag="a", name=f"A{chunk}")
            nc.vector.tensor_tensor(U[:], LO[:, :, 0:F - 3], LO[:, :, 3:F], MAX)
            nc.vector.tensor_tensor(A[:], U[:, :, 0:SC], LO[:, :, 6:F], MAX)

            V = sb.tile([128, CH, F - 3], F32, tag="uv", name=f"V{chunk}")
            Cc = sb.tile([128, CH, SC], F32, tag="c", name=f"Cc{chunk}")
            nc.vector.tensor_tensor(V[:], HI[:, :, 0:F - 3], HI[:, :, 3:F], MIN)
            nc.vector.tensor_tensor(Cc[:], V[:, :, 0:SC], HI[:, :, 6:F], MIN)

            Sm = sb.tile([128, CH, F - 3], F32, tag="sm", name=f"Sm{chunk}")
            Tm = sb.tile([128, CH, F - 3], F32, tag="tm", name=f"Tm{chunk}")
            nc.vector.tensor_tensor(Sm[:], T[:, :, 0:F - 3], T[:, :, 3:F], MIN)
            nc.vector.tensor_tensor(Tm[:], T[:, :, 0:F - 3], T[:, :, 3:F], MAX)
            # W (in Tm): min(Tm, MED+2)
            nc.vector.tensor_tensor(Tm[:, :, 0:SC], Tm[:, :, 0:SC], T[:, :, 6:F], MIN)
            # B (in Sm): max(Sm, W)
            nc.vector.tensor_tensor(Sm[:, :, 0:SC], Sm[:, :, 0:SC], Tm[:, :, 0:SC], MAX)

            # ---- final med3(A, B, C) -----------------------------------
            M1 = sb.tile([128, CH, SC], F32, tag="m1", bufs=2, name=f"M1{chunk}")
            nc.vector.tensor_tensor(M1[:], A[:], Sm[:, :, 0:SC], MIN)
            nc.vector.tensor_tensor(A[:], A[:], Sm[:, :, 0:SC], MAX)
            nc.vector.tensor_tensor(Cc[:], A[:], Cc[:], MIN)
            nc.vector.tensor_tensor(M1[:], M1[:], Cc[:], MAX)

            # ---- store -------------------------------------------------
            for ws in range(NW):
                for hs in range(NH):
                    p0 = ws * 32 + hs * 4
                    dst = AP(y.tensor,
                             (hs * HS + chunk * CH) * WC + ws * SC,
                             [[IMG, BPC], [WC, CH], [1, SC]])
                    nc.sync.dma_start(dst, M1[p0:p0 + 4, :, :])


def _build():
    if "nc" in _CACHE:
        return _CACHE["nc"]
    nc = bacc.Bacc("TRN2", target_bir_lowering=False, debug=False)
    x = nc.dram_tensor("x", [BPC, H, W, C], F32, kind="ExternalInput").ap()
    y = nc.dram_tensor("y", [BPC, H, W, C], F32, kind="ExternalOutput").ap()
    with tile.TileContext(nc) as tc:
        _build_kernel(tc, y, x)
    nc.compile()
    _CACHE["nc"] = nc
    return nc


def run(input_batch, **spmd_kwargs):
    nc = _build()
    in_maps = [
        {"x": np.ascontiguousarray(input_batch[i * BPC:(i + 1) * BPC])}
        for i in range(NCORES)
    ]
    res = run_bass_kernel_spmd(nc, in_maps, list(range(NCORES)), **spmd_kwargs)
    out = np.concatenate([r["y"] for r in res.results], axis=0)
    return out, res


def kernel(input_batch):
    out, _ = run(np.asarray(input_batch))
    return out


# revision 8
# speedup vs baseline: 1.6226x; 1.6226x over previous
"""3x3 median filter (reflect padding) on Trainium2, 8-core data parallel.

Layout (per core, 4 images):
  partition p = b*32 + g
    b in 0..3  : image index within the core's batch shard
    g in 0..31 : horizontal group of 7 consecutive output rows
  Work is split into NCHUNK width-chunks of CW output columns each.
  Each partition's slab holds 9 rows x (CW+2)px x 3ch fp32 in the free
  dimension, so vertical (stride F) and horizontal (stride 3) neighbor
  access are both free-dim offsets, and HBM rows transfer as long
  contiguous runs (good DMA efficiency).

Median of 9 = med3( max3(col_lows), med3(col_meds), min3(col_highs) )
with each vertical column triple sorted once and shared across the three
horizontally adjacent windows.

Loads/stores are split across the two hardware DGE queues (SP + ACT).
"""

import sys

if "/opt/trn_rl_repo" not in sys.path:
    sys.path.insert(0, "/opt/trn_rl_repo")

import numpy as np

import concourse.bass as bass  # noqa: F401
import concourse.tile as tile
from concourse import bacc, mybir
from concourse.ap import AP
from concourse.bass_utils import run_bass_kernel_spmd

F32 = mybir.dt.float32
MIN = mybir.AluOpType.min
MAX = mybir.AluOpType.max

B, H, W, C = 32, 224, 224, 3
NCORES = 8
BPC = B // NCORES   # 4 images per core
NG, GR = 32, 7      # row-groups per image, rows per group
NCHUNK, CW = 2, 112 # width chunks, output columns per chunk
SR = GR + 2         # slab rows incl. vertical halo
F = (CW + 2) * C    # 342 floats per slab row incl. horizontal halo
SC = CW * C         # 336 output floats per row-chunk
WC = W * C          # 672
IMG = H * WC

_CACHE = {}


def _build_kernel(tc, y, x):
    nc = tc.nc
    dma_engines = [nc.sync, nc.scalar]

    with tc.tile_pool(name="sb", bufs=1) as sb:
        for chunk in range(NCHUNK):
            S = sb.tile([128, SR, F], F32, tag="s", bufs=2, name=f"S{chunk}")

            # source float columns for this chunk (with horiz. clamping)
            c0 = chunk * CW
            so = c0 * 3 - 3
            fw, do = F, 0
            if chunk == 0:
                so, fw, do = 0, F - 3, 3
            if chunk == NCHUNK - 1:
                fw = F - 3

            # ---- loads: per image b, bulk + row-edge fixups ------------
            for b in range(BPC):
                eng = dma_engines[b % 2]
                p0 = b * 32
                base = b * IMG + so
                # bulk: groups 1..30 (full vertical halo available)
                src = AP(x.tensor, base + 6 * WC,
                         [[GR * WC, NG - 2], [WC, SR], [1, fw]])
                eng.dma_start(S[p0 + 1:p0 + 31, :, do:do + fw], src)
                # group 0: rows 0..7 into slab rows 1..8, reflect row 1
                e0 = AP(x.tensor, base, [[WC, SR - 1], [1, fw]])
                eng.dma_start(S[p0:p0 + 1, 1:SR, do:do + fw], e0)
                r0 = AP(x.tensor, base + WC, [[1, fw]])
                eng.dma_start(S[p0:p0 + 1, 0:1, do:do + fw], r0)
                # group 31: rows 216..223 into slab rows 0..7, reflect 222
                e1 = AP(x.tensor, base + (H - SR + 1) * WC,
                        [[WC, SR - 1], [1, fw]])
                eng.dma_start(S[p0 + 31:p0 + 32, 0:SR - 1, do:do + fw], e1)
                r1 = AP(x.tensor, base + (H - 2) * WC, [[1, fw]])
                eng.dma_start(S[p0 + 31:p0 + 32, SR - 1:SR, do:do + fw], r1)

            # horizontal reflect at the image borders
            if chunk == 0:
                nc.vector.tensor_copy(S[:, :, 0:3], S[:, :, 6:9])
            if chunk == NCHUNK - 1:
                nc.vector.tensor_copy(S[:, :, F - 3:F], S[:, :, F - 9:F - 6])

            # ---- stage 1: vertical column sort -------------------------
            P = sb.tile([128, GR, F], F32, tag="p", name=f"P{chunk}")
            Q = sb.tile([128, GR, F], F32, tag="q", name=f"Q{chunk}")
            nc.vector.tensor_tensor(P[:], S[:, 0:GR, :], S[:, 1:GR + 1, :], MIN)
            nc.vector.tensor_tensor(Q[:], S[:, 0:GR, :], S[:, 1:GR + 1, :], MAX)

            LO = sb.tile([128, GR, F], F32, tag="lo", name=f"LO{chunk}")
            T = sb.tile([128, GR, F], F32, tag="t", name=f"T{chunk}")
            nc.vector.tensor_tensor(LO[:], P[:, :, :], S[:, 2:SR, :], MIN)
            nc.vector.tensor_tensor(T[:], Q[:], S[:, 2:SR, :], MIN)
            # MED (in T): max(P, min(Q, S+2))
            nc.vector.tensor_tensor(T[:], P[:, :, :], T[:], MAX)
            # HI (in Q): max(Q, S+2)
            nc.vector.tensor_tensor(Q[:], Q[:], S[:, 2:SR, :], MAX)
            HI = Q

            # ---- stage 2: horizontal merge -----------------------------
            U = sb.tile([128, GR, F - 3], F32, tag="u", name=f"U{chunk}")
            nc.vector.tensor_tensor(U[:], LO[:, :, 0:F - 3], LO[:, :, 3:F], MAX)
            # A (in U): max3 of lows
            nc.vector.tensor_tensor(U[:, :, 0:SC], U[:, :, 0:SC],
                                    LO[:, :, 6:F], MAX)
            A = U

            V = sb.tile([128, GR, F - 3], F32, tag="v", name=f"V{chunk}")
            nc.vector.tensor_tensor(V[:], HI[:, :, 0:F - 3], HI[:, :, 3:F], MIN)
            # Cc (in V): min3 of highs
            nc.vector.tensor_tensor(V[:, :, 0:SC], V[:, :, 0:SC],
                                    HI[:, :, 6:F], MIN)
            Cc = V

            Sm = sb.tile([128, GR, F - 3], F32, tag="sm", name=f"Sm{chunk}")
            Tm = sb.tile([128, GR, F - 3], F32, tag="tm", name=f"Tm{chunk}")
            nc.vector.tensor_tensor(Sm[:], T[:, :, 0:F - 3], T[:, :, 3:F], MIN)
            nc.vector.tensor_tensor(Tm[:], T[:, :, 0:F - 3], T[:, :, 3:F], MAX)
            # W (in Tm): min(Tm, MED+2)
            nc.vector.tensor_tensor(Tm[:, :, 0:SC], Tm[:, :, 0:SC],
                                    T[:, :, 6:F], MIN)
            # B (in Sm): max(Sm, W) -> med3 of meds
            nc.vector.tensor_tensor(Sm[:, :, 0:SC], Sm[:, :, 0:SC],
                                    Tm[:, :, 0:SC], MAX)
            Bm = Sm

            # ---- final med3(A, B, C) -----------------------------------
            M1 = sb.tile([128, GR, SC], F32, tag="m1", bufs=2, name=f"M1{chunk}")
            nc.vector.tensor_tensor(M1[:], A[:, :, 0:SC], Bm[:, :, 0:SC], MIN)
            nc.vector.tensor_tensor(A[:, :, 0:SC], A[:, :, 0:SC],
                                    Bm[:, :, 0:SC], MAX)
            nc.vector.tensor_tensor(Cc[:, :, 0:SC], A[:, :, 0:SC],
                                    Cc[:, :, 0:SC], MIN)
            nc.vector.tensor_tensor(M1[:], M1[:], Cc[:, :, 0:SC], MAX)

            # ---- store -------------------------------------------------
            for b in range(BPC):
                eng = dma_engines[(b + 1) % 2]
                dst = AP(y.tensor, b * IMG + c0 * 3,
                         [[GR * WC, NG], [WC, GR], [1, SC]])
                eng.dma_start(dst, M1[b * 32:(b + 1) * 32, :, :])


def _build():
    if "nc" in _CACHE:
        return _CACHE["nc"]
    nc = bacc.Bacc("TRN2", target_bir_lowering=False, debug=False)
    x = nc.dram_tensor("x", [BPC, H, W, C], F32, kind="ExternalInput").ap()
    y = nc.dram_tensor("y", [BPC, H, W, C], F32, kind="ExternalOutput").ap()
    with tile.TileContext(nc) as tc:
        _build_kernel(tc, y, x)
    nc.compile()
    _CACHE["nc"] = nc
    return nc


def run(input_batch, **spmd_kwargs):
    nc = _build()
    in_maps = [
        {"x": np.ascontiguousarray(input_batch[i * BPC:(i + 1) * BPC])}
        for i in range(NCORES)
    ]
    res = run_bass_kernel_spmd(nc, in_maps, list(range(NCORES)), **spmd_kwargs)
    out = np.concatenate([r["y"] for r in res.results], axis=0)
    return out, res


def kernel(input_batch):
    out, _ = run(np.asarray(input_batch))
    return out
